# revision 9
# baseline (speedup 1.0000x reference)
"""Trainium2 Bass kernel v2 for the 2-layer GraphConv block.

  h1  = lrelu(segsum(x[src], dst) @ W1_rel.T + b1 + x @ W1_root.T)
  out = lrelu(segsum(h1[src], dst) @ W2_rel.T + b2 + h1 @ W2_root.T + x)

Strategy (dst-partitioned across 8 cores, one-hot matmul aggregation):
  - Each core owns 12500 dst nodes (padded to 12544 = 98*128 rows).
  - Edges sorted by (superchunk of dst, src-quarter, dst). Source rows are
    DGE-gathered (fp32, 256B rows) from one of 4 quarter tables of the full
    padded node table (quarter = 25088 rows, fits int16 idx range).
  - Aggregation = PE matmuls: aggT[64, 1024] += Xg_bf16[128e, 64].T-style
    one-hot products, accumulated in PSUM per superchunk (8 chunks of 128
    dst rows, spanning 2 PSUM banks with bank-grouped matmul ordering).
    One-hot O tiles are built on DVE by comparing a host-built rebased dst
    stream against an iota row (values pair-duplicated for the DVE 2x
    mode).  No DGE scatter, no DRAM agg.  Layer 1 skips the DGE gather
    entirely: its stream is host-materialized (bf16, pre-wrapped).
  - Dense phase fused per chunk: agg @ WrelT + root @ WrootT + bias
    (+ x residual) in PSUM, leaky-relu on the Act engine, write out.
  - AllGather (3.2MB/core) between layers for the halo exchange.
"""
import sys

sys.path.insert(0, '/opt/trn_rl_repo')

import numpy as np

N = 100000
D = 64
NCORES = 8
NPART = 12500                 # real nodes per core
NP = 12544                    # 98 * 128 padded part rows
NCHUNK = 98                   # 128-row chunks per core
SWIN = 8                      # chunks per superchunk (PSUM aggT spans 2 banks)
BANKW = 4                     # windows per PSUM bank ([64, 512] fp32)
NSUP = (NCHUNK + SWIN - 1) // SWIN     # 13 (last superchunk has 2 chunks)
QROWS = 2 * NP                # 25088 rows per gather quarter-table (< 2^15)
NQ = 4
NTOT = NCORES * NP            # 100352 padded full-table rows
OB = 16                       # one-hot instances per DVE batch op
PAB = 2                       # PSUM agg ring depth
SCRATCH = 98304               # dynamic_dma_scratch_size (6144-desc ring)
NSWQ = 2                      # SWDGE queues
GSTB = 5                      # gather staging ring depth
CAST_DVE = False              # cast gathered rows on DVE instead of Act
L1_ACT_COPIES = True          # route L1 psum->sbuf copies via Act engine
OTB = 4                       # one-hot tile ring depth
XGB = 2                       # per-superchunk Xg ring depth
NEG_SLOPE = 0.01
PAD_D = -20000                # dst marker for pad slots (never matches)


def _round128(n):
    return ((n + 127) // 128) * 128


def _set_config(npart):
    """Override the problem size (for small-scale simulator tests)."""
    global N, NPART, NP, NCHUNK, NSUP, QROWS, NTOT
    NPART = npart
    NP = _round128(npart)
    NCHUNK = NP // 128
    NSUP = (NCHUNK + SWIN - 1) // SWIN
    QROWS = 2 * NP
    NTOT = NCORES * NP
    N = NCORES * NPART


def _make_plan(src, dst):
    """Build the SPMD call structure + per-core index streams.

    Returns (plan, gstreams, dstrel) where
      plan: dict with call_len[S][q], call_start[S][q], L, insts (list per S
            of (g, win) instance tuples), inst_col (global column of each
            instance in the dst_rel tensor), maxcl, maxsl
      gstreams: [NCORES, L] int16 gather indices (quarter-local row ids)
      dstrel:   [NCORES, 128, NINST] int16 rebased dst streams
    """
    src = np.asarray(src, np.int64)
    dst = np.asarray(dst, np.int64)
    prow = (src // NPART) * NP + (src % NPART)   # padded global row
    q4 = prow // QROWS
    lidx = prow % QROWS
    p_of = dst // NPART
    dloc = dst % NPART
    S_of = dloc // (128 * SWIN)

    per_core = []
    seglen = np.zeros((NCORES, NSUP, NQ), np.int64)
    for p in range(NCORES):
        sel = p_of == p
        q = q4[sel]
        li = lidx[sel]
        dl = dloc[sel]
        s = S_of[sel]
        o = np.lexsort((dl, q, s))
        q, li, dl, s = q[o], li[o], dl[o], s[o]
        np.add.at(seglen, (p, s, q), 1)
        per_core.append((q, li, dl, s))

    call_len = np.zeros((NSUP, NQ), np.int64)
    for S in range(NSUP):
        for qq in range(NQ):
            m = seglen[:, S, qq].max()
            if m > 0:
                call_len[S, qq] = _round128(m)
    call_start = np.zeros((NSUP, NQ), np.int64)
    pos = 0
    for S in range(NSUP):
        for qq in range(NQ):
            call_start[S, qq] = pos
            pos += call_len[S, qq]
    L = pos

    # per-core flat streams
    gstreams = np.zeros((NCORES, L), np.int16)
    dstloc_all = np.full((NCORES, L), PAD_D, np.int32)
    for p in range(NCORES):
        q, li, dl, s = per_core[p]
        # edges already sorted by (s, q, dl); place each (s, q) run at its
        # call_start
        key = s * NQ + q
        nb = np.r_[True, key[1:] != key[:-1]] if len(key) else np.zeros(0, bool)
        gid = np.cumsum(nb) - 1
        st = np.flatnonzero(nb)
        off = np.arange(len(key)) - st[gid] if len(key) else np.zeros(0, np.int64)
        base = call_start[s[st], q[st]] if len(st) else np.zeros(0, np.int64)
        posn = base[gid] + off
        gs = np.zeros(L, np.int64)
        da = np.full(L, PAD_D, np.int64)
        gs[posn] = li
        da[posn] = dl
        gstreams[p] = gs.astype(np.int16)
        dstloc_all[p] = da.astype(np.int32)

    # group -> superchunk map
    ngroups = L // 128
    group_S = np.zeros(ngroups, np.int64)
    for S in range(NSUP):
        for qq in range(NQ):
            a, l = call_start[S, qq], call_len[S, qq]
            group_S[a // 128:(a + l) // 128] = S

    # instances: per group, union over cores of touched windows
    win_all = np.where(dstloc_all >= 0, dstloc_all // 128, -1)  # [NCORES, L]
    insts = [[] for _ in range(NSUP)]
    for g in range(ngroups):
        wset = np.unique(win_all[:, g * 128:(g + 1) * 128])
        wset = wset[wset >= 0]
        S = group_S[g]
        for w in wset:
            insts[S].append((g, int(w)))
    # coverage: every window of every superchunk needs >= 1 instance
    for S in range(NSUP):
        covered = {w for (_, w) in insts[S]}
        wlo = S * SWIN
        whi = min(wlo + SWIN, NCHUNK)
        # pick any group of this superchunk for dummies
        gS = np.flatnonzero(group_S == S)
        g0 = int(gS[0]) if len(gS) else 0
        for w in range(wlo, whi):
            if w not in covered:
                insts[S].append((g0, w))
        # order by (PSUM bank, group, window): matmul start=True clears a
        # whole bank, so each bank's instances must be contiguous with
        # start on the first and stop on the last
        insts[S].sort(key=lambda t: ((t[1] - wlo) // BANKW, t[0], t[1]))

    # instance columns (global, S-major emission order) + per-bank
    # start/stop flags
    inst_col = {}
    inst_flags = [[] for _ in range(NSUP)]
    ninst = 0
    for S in range(NSUP):
        wlo = S * SWIN
        banks = [(t[1] - wlo) // BANKW for t in insts[S]]
        for j, t in enumerate(insts[S]):
            first = j == 0 or banks[j] != banks[j - 1]
            last = j == len(banks) - 1 or banks[j] != banks[j + 1]
            inst_flags[S].append((first, last))
            inst_col[(S,) + t] = ninst
            ninst += 1

    # dst_rel per core: [128, NINST, 2] int16 (each value duplicated as an
    # adjacent pair so the DVE one-hot compare runs in its 2x mode)
    dstrel = np.zeros((NCORES, 128, ninst, 2), np.int16)
    for p in range(NCORES):
        da = dstloc_all[p].reshape(ngroups, 128)   # [g, slot]
        for S in range(NSUP):
            for (g, w) in insts[S]:
                col = inst_col[(S, g, w)]
                v = da[g].astype(np.int64) - 128 * w
                v = np.clip(v, -32000, 32000)
                dstrel[p, :, col, 0] = v.astype(np.int16)
                dstrel[p, :, col, 1] = v.astype(np.int16)
    dstrel = dstrel.reshape(NCORES, 128, ninst * 2)

    # ---- layer-1 window-aligned plan (stream is host-materialized, so
    # there is no quarter constraint and pads cost only stream bytes;
    # aligning groups to windows makes exactly one instance per group) ----
    NW = NCHUNK
    wcnt = np.zeros((NCORES, NW), np.int64)
    pc1 = []
    for p in range(NCORES):
        sel = p_of == p
        dl = dloc[sel]
        pr = prow[sel]
        o = np.argsort(dl, kind='stable')
        dl, pr = dl[o], pr[o]
        np.add.at(wcnt, (p, dl // 128), 1)
        pc1.append((dl, pr))
    wlen = np.zeros(NW, np.int64)
    for w in range(NW):
        m = wcnt[:, w].max()
        wlen[w] = _round128(m) if m > 0 else 128
    wstart = np.zeros(NW, np.int64)
    wstart[1:] = np.cumsum(wlen)[:-1]
    L1 = int(wlen.sum())
    prow1 = np.zeros((NCORES, L1), np.int64)
    drel1v = np.full((NCORES, L1), PAD_D, np.int64)
    for p in range(NCORES):
        dl, pr = pc1[p]
        w = dl // 128
        nbw = np.r_[True, w[1:] != w[:-1]] if len(w) else np.zeros(0, bool)
        gidw = np.cumsum(nbw) - 1
        stw = np.flatnonzero(nbw)
        off = np.arange(len(w)) - stw[gidw] if len(w) else np.zeros(0, np.int64)
        posn = (wstart[w[stw]][gidw] + off) if len(stw) else np.zeros(0, np.int64)
        prow1[p, posn] = pr
        drel1v[p, posn] = dl - 128 * w
    ngroups1 = L1 // 128
    g_w1 = np.repeat(np.arange(NW), wlen // 128)
    insts1 = [[] for _ in range(NSUP)]
    flags1 = [[] for _ in range(NSUP)]
    for g in range(ngroups1):
        insts1[int(g_w1[g]) // SWIN].append((g, int(g_w1[g])))
    for S in range(NSUP):
        wlo = S * SWIN
        banks = [(t[1] - wlo) // BANKW for t in insts1[S]]
        for j in range(len(banks)):
            first = j == 0 or banks[j] != banks[j - 1]
            last = j == len(banks) - 1 or banks[j] != banks[j + 1]
            flags1[S].append((first, last))
    drel1 = np.zeros((NCORES, 128, ngroups1, 2), np.int16)
    for p in range(NCORES):
        v = np.clip(drel1v[p].reshape(ngroups1, 128).T, -32000, 32000)
        drel1[p, :, :, 0] = v.astype(np.int16)
        drel1[p, :, :, 1] = v.astype(np.int16)
    drel1 = drel1.reshape(NCORES, 128, ngroups1 * 2)
    scols1 = [int(wlen[S * SWIN:min(S * SWIN + SWIN, NW)].sum()) // 128
              for S in range(NSUP)]
    S0_1 = [int(wstart[S * SWIN]) for S in range(NSUP)]

    maxcl = int(call_len.max())
    maxsl = max(int(call_len.sum(axis=1).max()),
                max(sc * 128 for sc in scols1))
    plan = dict(call_len=call_len, call_start=call_start, L=L,
                insts=insts, inst_col=inst_col, ninst=ninst,
                inst_flags=inst_flags,
                maxcl=maxcl, maxsl=maxsl, group_S=group_S,
                pads=(dstloc_all == PAD_D),
                L1=L1, insts1=insts1, flags1=flags1, ninst1=ngroups1,
                scols1=scols1, S0_1=S0_1, prow1=prow1,
                pads1=(drel1v == PAD_D))
    return plan, gstreams, dstrel, drel1


def _wrap_stream(a):
    """[L] int16 -> [128, L//16] wrapped (idx i at [i%16, i//16]), repl 8x."""
    L = len(a)
    assert L % 16 == 0
    w = a.reshape(L // 16, 16).T
    return np.tile(w, (8, 1)).copy()


MAXCALL = 1024                # max gather-call idxs (HW DGE limit: >1024 fails)
ABLATE = set()                # timing experiments: {'gather','ogen','mm','coll','dense'}


def _gather_calls(plan):
    """Flatten (S, q4) segments into gather calls of <= MAXCALL idxs.

    Returns per-S list of (q4, stream_start, length)."""
    out = []
    for S in range(NSUP):
        cs = []
        for q in range(NQ):
            a = int(plan["call_start"][S, q])
            l = int(plan["call_len"][S, q])
            while l > 0:
                ln = min(MAXCALL, l)
                cs.append((q, a, ln))
                a += ln
                l -= ln
        out.append(cs)
    return out


def _build_nc(plan):
    from concourse import tile, mybir, masks
    import concourse.bacc as bacc

    f32 = mybir.dt.float32
    bf16 = mybir.dt.bfloat16
    i16 = mybir.dt.int16
    AF = mybir.ActivationFunctionType

    L = plan["L"]
    cols = L // 16
    ninst = plan["ninst"]
    maxcl = min(plan["maxcl"], MAXCALL)
    maxsl = plan["maxsl"]
    calls = _gather_calls(plan)

    nc = bacc.Bacc(None, target_bir_lowering=False, num_devices=NCORES,
                   dynamic_dma_scratch_size=SCRATCH, num_swdge_queues=NSWQ)

    L1 = plan["L1"]
    ninst1 = plan["ninst1"]
    xs1_in = nc.declare_dram_parameter("xs1", [128, (L1 // 128) * D], bf16,
                                       isOutput=False)
    xT_in = nc.declare_dram_parameter("xT", [D, NP], bf16, isOutput=False)
    w_ins = {nm: nc.declare_dram_parameter(nm, [D, D], bf16, isOutput=False)
             for nm in ["W1relT", "W1rootT", "W2relT", "W2rootT", "I64"]}
    b_ins = {nm: nc.declare_dram_parameter(nm, [1, D], bf16, isOutput=False)
             for nm in ["b1", "b2"]}
    gidx_in = nc.declare_dram_parameter("gidx", [128, cols], i16, isOutput=False)
    drel_in = nc.declare_dram_parameter("drel", [128, ninst * 2], i16,
                                        isOutput=False)
    drel1_in = nc.declare_dram_parameter("drel1", [128, ninst1 * 2], i16,
                                         isOutput=False)
    iota_in = nc.declare_dram_parameter("iota16", [128, 128], i16,
                                        isOutput=False)
    y_out = nc.declare_dram_parameter("y", [NP, D], f32, isOutput=True)

    h1_dram = nc.dram_tensor("h1_dram", [NP, D], f32)
    h_full = nc.dram_tensor("h_full", [NTOT, D], f32, addr_space="Shared")

    with tile.TileContext(nc) as tc:
        with (
            tc.tile_pool(name="const", bufs=1) as cpool,
            tc.tile_pool(name="gst", bufs=GSTB) as gpool,
            tc.tile_pool(name="xg", bufs=XGB) as xpool,
            tc.tile_pool(name="o", bufs=OTB) as opool,
            tc.tile_pool(name="mm", bufs=3) as mpool,
            tc.tile_pool(name="psum", bufs=2, space="PSUM") as ppool2,
            tc.tile_pool(name="psumA", bufs=PAB, space="PSUM") as ppoolA,
        ):
            # ---- constants ----
            ident = cpool.tile([128, 128], f32, tag="ident")
            masks.make_identity(nc, ident[:])
            ones1 = cpool.tile([1, 128], bf16, tag="ones1")
            nc.gpsimd.memset(ones1[:], 1.0)
            iota16 = cpool.tile([128, 128], i16, tag="iota16")
            nc.sync.dma_start(iota16[:], iota_in[:])
            wt = {}
            for nm, t_in in w_ins.items():
                t = cpool.tile([D, D], bf16, tag=nm, name=nm)
                nc.sync.dma_start(t[:], t_in[:])
                wt[nm] = t
            bt = {}
            for nm, t_in in b_ins.items():
                t = cpool.tile([1, D], bf16, tag=nm, name=nm)
                nc.sync.dma_start(t[:], t_in[:])
                bt[nm] = t
            h1T_sb = cpool.tile([D, NP], bf16, tag="h1T_sb")
            drel = cpool.tile([128, ninst * 2], i16, tag="drel")
            nc.sync.dma_start(drel[:], drel_in[:])
            drel1 = cpool.tile([128, ninst1 * 2], i16, tag="drel1")
            nc.sync.dma_start(drel1[:], drel1_in[:])

            qi = [0]
            if CAST_DVE:
                def _cast(o, i):
                    nc.vector.tensor_copy(o, i)
            else:
                def _cast(o, i):
                    nc.scalar.copy(o, i)

            def do_layer(tables, wrel, wroot, bias, root_sb, resid,
                         out_dram, save_h1T):
                for S in range(NSUP):
                    if tables is None:
                        S0 = plan["S0_1"][S]
                        ilist = plan["insts1"][S]
                        iflags = plan["flags1"][S]
                        scols = plan["scols1"][S]
                        base_col = ilist[0][0] if ilist else 0
                        dreli = drel1
                    else:
                        S0 = int(plan["call_start"][S, 0])
                        ilist = plan["insts"][S]
                        iflags = plan["inst_flags"][S]
                        scols = sum(cl for (_, _, cl) in calls[S]) // 128
                        base_col = (plan["inst_col"][(S,) + ilist[0]]
                                    if ilist else 0)
                        dreli = drel
                    nI = len(ilist)
                    if nI == 0:
                        continue

                    xg = xpool.tile([128, maxsl // 128, D], bf16, tag="xg")
                    if tables is None:
                        # layer 1: host-materialized bf16 gather stream
                        c0s = S0 // 128
                        nc.sync.dma_start(
                            xg[:, :scols, :],
                            xs1_in[:, c0s * D:(c0s + scols) * D].rearrange(
                                "p (c d) -> p c d", d=D))
                    else:
                        # stream this superchunk's gather idxs from DRAM
                        gxs = xpool.tile([128, (maxsl + 2047) // 16], i16,
                                         tag="gxs", bufs=3)
                        nc.sync.dma_start(
                            gxs[:, :scols * 8],
                            gidx_in[:, S0 // 16:S0 // 16 + scols * 8])
                        for (q4, cs, cl) in calls[S]:
                            if 'gather' in ABLATE:
                                break
                            gst = gpool.tile([128, maxcl // 128, D], f32,
                                             tag="gst")
                            nc.gpsimd.dma_gather(
                                gst[:, :cl // 128, :], tables[q4],
                                gxs[:, (cs - S0) // 16:(cs - S0 + cl) // 16],
                                cl, cl, D, queue_num=qi[0] % NSWQ)
                            qi[0] += 1
                            co = (cs - S0) // 128
                            _cast(xg[:, co:co + cl // 128, :],
                                  gst[:, :cl // 128, :])

                    if 'agg' in ABLATE:
                        continue
                    psA = ppoolA.tile([D, SWIN * 128], f32, tag="agg")
                    for b0 in range(0, nI, OB):
                        bn = min(OB, nI - b0)
                        ot = opool.tile([128, OB, 128], bf16, tag="o")
                        c0 = base_col + b0
                        if 'ogen' not in ABLATE:
                            nc.vector.tensor_tensor(
                            ot[:, :bn, :].rearrange("p b (j k) -> p b j k",
                                                    k=2),
                            dreli[:, 2 * c0:2 * (c0 + bn)].rearrange(
                                "p (b o k) -> p b o k", o=1,
                                k=2).broadcast_to([128, bn, 64, 2]),
                            iota16[:].rearrange(
                                "p (o j k) -> p o j k", o=1,
                                k=2).broadcast_to([128, bn, 64, 2]),
                            mybir.AluOpType.is_equal)
                        for j in range(bn):
                            if 'mm' in ABLATE:
                                break
                            g, w = ilist[b0 + j]
                            wi = w - S * SWIN
                            fst, lst = iflags[b0 + j]
                            nc.tensor.matmul(
                                psA[:, wi * 128:(wi + 1) * 128],
                                xg[:, g - S0 // 128, :], ot[:, j, :],
                                start=fst, stop=lst)

                    nwin = min(SWIN, NCHUNK - S * SWIN)
                    aggT = mpool.tile([D, SWIN * 128], bf16, tag="aggT")
                    if tables is None and L1_ACT_COPIES:
                        nc.scalar.copy(aggT[:, :nwin * 128],
                                       psA[:, :nwin * 128])
                    else:
                        nc.vector.tensor_copy(aggT[:, :nwin * 128],
                                              psA[:, :nwin * 128])

                    if 'dense' in ABLATE:
                        continue
                    if root_sb is None or resid:
                        # one bulk xT slice per superchunk (root in L1,
                        # residual in L2) instead of per-chunk loads
                        xts = mpool.tile([D, SWIN * 128], bf16, tag="xts")
                        nc.sync.dma_start(
                            xts[:, :nwin * 128],
                            xT_in[:, S * SWIN * 128:
                                  S * SWIN * 128 + nwin * 128])
                    for w in range(S * SWIN, min(S * SWIN + SWIN, NCHUNK)):
                        wi = w - S * SWIN
                        ps_o = ppool2.tile([128, D], f32, tag="dense")
                        nc.tensor.matmul(ps_o[:],
                                         aggT[:, wi * 128:(wi + 1) * 128],
                                         wrel[:], start=True, stop=False)
                        if root_sb is None:
                            root = xts[:, wi * 128:(wi + 1) * 128]
                        else:
                            root = root_sb[:, w * 128:(w + 1) * 128]
                        nc.tensor.matmul(ps_o[:], root,
                                         wroot[:], start=False, stop=False)
                        if resid:
                            nc.tensor.matmul(ps_o[:],
                                             xts[:, wi * 128:(wi + 1) * 128],
                                             wt["I64"][:], start=False,
                                             stop=False)
                        nc.tensor.matmul(ps_o[:], ones1[:], bias[:],
                                         start=False, stop=True)
                        tmp = mpool.tile([128, D], f32, tag="tmp")
                        nc.scalar.mul(tmp[:], ps_o[:], NEG_SLOPE)
                        hst = mpool.tile([128, D], f32, tag="hst")
                        nc.vector.tensor_max(hst[:], ps_o[:], tmp[:])
                        nc.sync.dma_start(out_dram[w * 128:(w + 1) * 128, :],
                                          hst[:])
                        if save_h1T:
                            ps_t = ppool2.tile([D, 128], f32, tag="tr")
                            nc.tensor.transpose(ps_t[:], hst[:], ident[:])
                            if L1_ACT_COPIES:
                                nc.scalar.copy(
                                    h1T_sb[:, w * 128:(w + 1) * 128],
                                    ps_t[:])
                            else:
                                nc.vector.tensor_copy(
                                    h1T_sb[:, w * 128:(w + 1) * 128],
                                    ps_t[:])

            # ================= layer 1 =================
            do_layer(None, wt["W1relT"], wt["W1rootT"], bt["b1"],
                     None, False, h1_dram, True)

            # ================= halo exchange =================
            if 'coll' not in ABLATE:
                nc.gpsimd.collective_compute(
                    "AllGather", mybir.AluOpType.bypass,
                    replica_groups=[list(range(NCORES))],
                    ins=[h1_dram[:].opt()], outs=[h_full[:].opt()])

            # ================= layer 2 =================
            if 'l2' not in ABLATE:
                h_tabs = [h_full[q * QROWS:(q + 1) * QROWS, :]
                          for q in range(NQ)]
                do_layer(h_tabs, wt["W2relT"], wt["W2rootT"], bt["b2"],
                         h1T_sb, True, y_out, False)

    nc.compile()
    return nc


def _prep_inputs(x, edge_index, W1_rel, b1, W1_root, W2_rel, b2, W2_root):
    import ml_dtypes
    bf = ml_dtypes.bfloat16

    src = np.asarray(edge_index[0]).astype(np.int64)
    dst = np.asarray(edge_index[1]).astype(np.int64)
    plan, gstreams, dstrel, drel1 = _make_plan(src, dst)

    x = np.asarray(x, np.float32)
    x_full = np.zeros((NTOT, D), np.float32)
    for p in range(NCORES):
        x_full[p * NP:p * NP + NPART] = x[p * NPART:(p + 1) * NPART]

    common = {}
    common["W1relT"] = np.ascontiguousarray(np.asarray(W1_rel).T).astype(bf)
    common["W1rootT"] = np.ascontiguousarray(np.asarray(W1_root).T).astype(bf)
    common["W2relT"] = np.ascontiguousarray(np.asarray(W2_rel).T).astype(bf)
    common["W2rootT"] = np.ascontiguousarray(np.asarray(W2_root).T).astype(bf)
    common["I64"] = np.eye(D, dtype=bf)
    common["b1"] = np.asarray(b1, np.float32).reshape(1, D).astype(bf)
    common["b2"] = np.asarray(b2, np.float32).reshape(1, D).astype(bf)
    common["iota16"] = np.tile(np.arange(128, dtype=np.int16), (128, 1))

    in_maps = []
    for p in range(NCORES):
        m = dict(common)
        xT = np.zeros((D, NP), np.float32)
        xT[:, :NPART] = x[p * NPART:(p + 1) * NPART].T
        m["xT"] = xT.astype(bf)
        m["gidx"] = _wrap_stream(gstreams[p])
        m["drel"] = np.ascontiguousarray(dstrel[p])
        m["drel1"] = np.ascontiguousarray(drel1[p])
        L1 = plan["L1"]
        rows = x_full[plan["prow1"][p]].astype(bf)
        rows[plan["pads1"][p]] = 0
        m["xs1"] = np.ascontiguousarray(
            rows.reshape(L1 // 128, 128, D).transpose(1, 0, 2).reshape(
                128, (L1 // 128) * D))
        in_maps.append(m)
    return plan, in_maps


def kernel(x, edge_index, W1_rel, b1, W1_root, W2_rel, b2, W2_root):
    from concourse import bass_utils

    plan, in_maps = _prep_inputs(x, edge_index, W1_rel, b1, W1_root,
                                 W2_rel, b2, W2_root)
    nc = _build_nc(plan)
    res = bass_utils.run_bass_kernel_spmd(nc, in_maps,
                                          core_ids=list(range(NCORES)))
    out = np.concatenate([res.results[p]["y"][:NPART]
                          for p in range(NCORES)], 0)
    return out.astype(np.float32)


def _emulate_agg(plan, gstreams, dstrel, table, p):
    """Numpy emulation of the on-device aggregation for core p.

    table: [NTOT, D] padded node features. Returns agg [NP, D] float64.
    """
    call_len, call_start = plan["call_len"], plan["call_start"]
    agg = np.zeros((NP, D), np.float64)
    gs = gstreams[p].astype(np.int64)
    for S in range(NSUP):
        # Xg for this superchunk, indexed by (global group - S first group)
        for (g, w) in plan["insts"][S]:
            col = plan["inst_col"][(S, g, w)]
            # which call does group g belong to?
            qq = None
            for q in range(NQ):
                a, l = call_start[S, q], call_len[S, q]
                if a <= g * 128 < a + l:
                    qq = q
                    break
            assert qq is not None
            rows = gs[g * 128:(g + 1) * 128] + qq * QROWS
            Xg = table[rows]                       # [128, D]
            dr = dstrel[p, :, col].astype(np.int64)
            O = (dr[:, None] == np.arange(128)[None, :]).astype(np.float64)
            agg[w * 128:(w + 1) * 128] += O.T @ Xg
    return agg


if __name__ == "__main__":
    rng = np.random.default_rng(0)
    E = 400000
    src = rng.integers(0, N, E)
    dst = rng.integers(0, N, E)
    plan, gstreams, dstrel = _make_plan(src, dst)
    cl = plan["call_len"]
    print(f"L={plan['L']} ({plan['L']/ (E/8):.3f}x of E/8)  ninst={plan['ninst']}"
          f" maxcl={plan['maxcl']} maxsl={plan['maxsl']}")
    x = rng.normal(size=(N, D)).astype(np.float32)
    table = np.zeros((NTOT, D), np.float32)
    for p in range(NCORES):
        table[p * NP:p * NP + NPART] = x[p * NPART:(p + 1) * NPART]
    for p in range(2):
        agg = _emulate_agg(plan, gstreams, dstrel, table, p)
        sel = (dst >= p * NPART) & (dst < (p + 1) * NPART)
        ref = np.zeros((NPART, D), np.float64)
        np.add.at(ref, dst[sel] - p * NPART, x[src[sel]])
        err = np.abs(agg[:NPART] - ref).max()
        print(f"core {p}: emulated agg err {err:.3e}")
